# revision 17
# baseline (speedup 1.0000x reference)
"""Distributed GQA attention kernel for Trainium2 (8 NeuronCores).

Problem: nn_Attention_69595650065097
  B=2, S=2048, D=4096, H=32 q-heads, KV=8 kv-heads, HD=128.
  reference returns (out[B,S,D], xk[B,S,KV,HD], xv[B,S,KV,HD]).

Sharding: tensor-parallel over heads. Each of the 8 cores owns 4 q-heads and
1 kv-head, computes attention for them over the full sequence, then an
AllToAll redistributes attention outputs so each core owns a 1/8 slice of the
sequence with the full hidden dim for the output projection (full wo on every
core, streamed from HBM).

Numerics: all matmuls run in bf16 (4x faster on the PE than f32) unless a
host-side estimate of the score magnitude says bf16 score error would corrupt
the softmax, in which case the Q/K path (projections + scores) runs in f32.

Host-side layout tricks folded into the shards (free):
  - RoPE de-interleave permutation baked into wq/wk rows, so on-device rope is
    contiguous-half form; a permutation matrix in the xk transpose undoes it.
  - 1/sqrt(HD) folded into wq.
  - x, fr, fi, wo pre-transposed to the layouts the device wants.
"""

import numpy as np
import ml_dtypes

import concourse.bass as bass
import concourse.mybir as mybir
import concourse.tile as tile
from concourse import bacc
from concourse.bass import ds
from concourse.bass_utils import run_bass_kernel_spmd

B, S, D, H, KV, HD = 2, 2048, 4096, 32, 8, 128
N_CORES = 8
HPC = H // N_CORES            # q heads per core = 4
T = B * S                     # tokens = 4096
TC = 512                      # token chunk for projections
NTC = T // TC                 # 8
ND = D // 128                 # 32 d-tiles
SC = S // N_CORES             # 256: seq slice per core after A2A
TOKL = B * SC                 # 512 local tokens for wo
QB = 512                      # q block in attention
NQB = S // QB                 # 4
KT = 128                      # k tile
KC = 512                      # k chunk (4 k tiles)

F32 = mybir.dt.float32
BF16 = mybir.dt.bfloat16
BF16_NP = ml_dtypes.bfloat16


# ---------------------------------------------------------------- host prep

def _deinterleave_perm():
    return np.concatenate([np.arange(0, HD, 2), np.arange(1, HD, 2)])


def _classify_mask(mask):
    """mask: [S, S] f32. 128x128 blocks: 0=zero, 1=neginf, 2=mixed.
    Returns (blocks[16,16] int8, jlim[NQB]: #512-chunks to compute/q-block)."""
    nq = S // 128
    m = mask.reshape(nq, 128, nq, 128)
    bmax = m.max(axis=(1, 3))
    bmin = m.min(axis=(1, 3))
    blocks = np.full((nq, nq), 2, np.int8)
    blocks[(bmax == 0) & (bmin == 0)] = 0
    blocks[bmax <= -1e8] = 1
    jlim = np.zeros(NQB, np.int64)
    for qb in range(NQB):
        lim = 0
        for qt in range(4 * qb, 4 * qb + 4):
            nonneg = np.nonzero(blocks[qt] != 1)[0]
            if len(nonneg):
                lim = max(lim, int(nonneg[-1]) // 4 + 1)
        jlim[qb] = lim
    return blocks, jlim


def _estimate_score_max(x, wq_scaled_perm, wk_perm, fr, fi):
    """Cheap host-side estimate of max |score| using sampled tokens."""
    rng = np.random.default_rng(1234)
    qs = np.sort(rng.choice(S, 32, replace=False))
    ks = np.sort(rng.choice(S, 256, replace=False))

    def proj_rope(tok_idx, w, nh):
        v = x[0, tok_idx].astype(np.float64) @ w.astype(np.float64).T
        v = v.reshape(len(tok_idx), nh, HD)
        lo, hi = v[..., :64], v[..., 64:]
        f_r = fr[tok_idx][:, None, :]
        f_i = fi[tok_idx][:, None, :]
        return np.concatenate([lo * f_r - hi * f_i, hi * f_r + lo * f_i], -1)

    q = proj_rope(qs, wq_scaled_perm, H)
    k = proj_rope(ks, wk_perm, KV)
    kk = np.repeat(k, H // KV, axis=1)
    s = np.einsum("qhd,khd->hqk", q, kk)
    return float(np.abs(s).max())


def _estimate_rowmax_min(x, wq_scaled_perm, wk_perm, fr, fi, mask):
    """Estimate min over rows of the row-max score (sampled, mask-aware)."""
    rng = np.random.default_rng(99)
    qs = np.sort(rng.choice(S, 48, replace=False))
    ks = np.sort(np.unique(np.concatenate([qs, rng.choice(S, 192,
                                                          replace=False)])))

    def proj_rope(tok_idx, w, nh, b):
        v = x[b, tok_idx].astype(np.float64) @ w.astype(np.float64).T
        v = v.reshape(len(tok_idx), nh, HD)
        lo, hi = v[..., :64], v[..., 64:]
        f_r = fr[tok_idx][:, None, :]
        f_i = fi[tok_idx][:, None, :]
        return np.concatenate([lo * f_r - hi * f_i, hi * f_r + lo * f_i], -1)

    worst = np.inf
    for b in range(B):
        q = proj_rope(qs, wq_scaled_perm, H, b)
        k = proj_rope(ks, wk_perm, KV, b)
        kk = np.repeat(k, H // KV, axis=1)
        sc = np.einsum("qhd,khd->hqk", q, kk)
        sc = sc + mask[np.ix_(qs, ks)][None]
        rm = sc.max(axis=2)          # [h, q] row maxes over sampled k
        valid = (mask[np.ix_(qs, ks)].max(axis=1) > -1e8)
        if valid.any():
            worst = min(worst, float(rm[:, valid].min()))
    return worst


# ---------------------------------------------------------------- device IR

def build_program(qk_f32, blocks, jlim, mask_f32, exp_shift):
    """Build the SPMD Bass program (identical on all 8 cores)."""
    QKDT = F32 if qk_f32 else BF16
    MDT = F32 if mask_f32 else BF16

    nc = bacc.Bacc("TRN2", target_bir_lowering=False, debug=False,
                   num_devices=N_CORES)

    # ---- I/O ----
    xT = nc.dram_tensor("xT", [D, T], QKDT, kind="ExternalInput")
    xTv = None
    if qk_f32:
        xTv = nc.dram_tensor("xTv", [D, T], BF16, kind="ExternalInput")
    wqT = nc.dram_tensor("wqT", [D, HPC * HD], QKDT, kind="ExternalInput")
    wkT = nc.dram_tensor("wkT", [D, HD], QKDT, kind="ExternalInput")
    wvT = nc.dram_tensor("wvT", [D, HD], BF16, kind="ExternalInput")
    woT = nc.dram_tensor("woT", [H * HD, D], BF16, kind="ExternalInput")
    frT = nc.dram_tensor("frT", [64, S], F32, kind="ExternalInput")
    fiT = nc.dram_tensor("fiT", [64, S], F32, kind="ExternalInput")
    mixed_list = [(qt, kt) for qt in range(16) for kt in range(16)
                  if blocks[qt, kt] == 2]
    maskb = None
    if mixed_list:
        maskb = nc.dram_tensor("maskb", [len(mixed_list), 128, 128], MDT,
                               kind="ExternalInput")
    mixed_idx = {qk: i for i, qk in enumerate(mixed_list)}

    out_d = nc.dram_tensor("out", [TOKL, D], F32, kind="ExternalOutput")
    xk_d = nc.dram_tensor("xk", [T, HD], F32, kind="ExternalOutput")
    xv_d = nc.dram_tensor("xv", [T, HD], F32, kind="ExternalOutput")

    perm = _deinterleave_perm()
    perm_mat = np.zeros((HD, HD), np.float32)
    perm_mat[np.arange(HD), perm] = 1.0
    permM_d = nc.inline_tensor(perm_mat, "permM")
    ident_d = nc.inline_tensor(np.eye(HD, dtype=np.float32), "identM")
    identb_d = nc.inline_tensor(np.eye(HD, dtype=BF16_NP), "identMb")

    with tile.TileContext(nc) as tc:
        with (
            tc.tile_pool(name="const", bufs=1) as constp,
            tc.tile_pool(name="dram", bufs=1, space="DRAM") as dram,
        ):
            # ---- constants ----
            frs = constp.tile([128, S], F32, tag="frs")
            fis = constp.tile([128, S], F32, tag="fis")
            nc.sync.dma_start(frs[0:64, :], frT[:])
            nc.sync.dma_start(frs[64:128, :], frT[:])
            nc.sync.dma_start(fis[0:64, :], fiT[:])
            nc.sync.dma_start(fis[64:128, :], fiT[:])
            permM = constp.tile([HD, HD], F32, tag="permM")
            nc.sync.dma_start(permM[:], permM_d[:])
            identM = constp.tile([HD, HD], F32, tag="identM")
            nc.sync.dma_start(identM[:], ident_d[:])
            identMb = constp.tile([HD, HD], BF16, tag="identMb")
            nc.sync.dma_start(identMb[:], identb_d[:])
            shiftb = None
            if exp_shift is not None:
                shiftb = constp.tile([128, 1], F32, tag="shiftb")
                nc.vector.memset(shiftb[:], -float(exp_shift))


            # ---- A2A bounce buffers ----
            a2a_in = [dram.tile([N_CORES, HD, B, SC], BF16,
                                name=f"a2ain{h}") for h in range(HPC)]
            a2a_out = [dram.tile([N_CORES, HD, B, SC], BF16,
                                 name=f"a2aout{h}") for h in range(HPC)]

            # ---- residents for phases B+C (freed before phase D) ----
            qkv_ctx = tc.tile_pool(name="qkv", bufs=1)
            qkvp = qkv_ctx.__enter__()
            qts = [qkvp.tile([HD, T], QKDT, tag=f"qt{h}", name=f"qt{h}")
                   for h in range(HPC)]
            kf = (qkvp.tile([HD, T], F32, tag="kf", name="kf")
                  if qk_f32 else None)
            ktt = (kf if qk_f32 else
                   qkvp.tile([HD, T], BF16, tag="ktt", name="ktt"))
            vtok = qkvp.tile([128, T], BF16, tag="vtok")  # V token-major

            # ================= Phase B: projections + rope =================
            # In f32 QK mode SBUF cannot hold wq(f32) + wk + wv at once, so
            # run two passes (Q-only, then K+V), streaming xT twice.
            with (
                tc.tile_pool(name="xin", bufs=(3 if qk_f32 else 6)) as xinp,
                tc.tile_pool(name="ropet", bufs=1) as ropet,
                            ):

                def rope(ps, s0, o_lo, o_hi, f_lo=None, f_hi=None):
                    fr_lo = frs[0:64, ds(s0, TC)]
                    fr_hi = frs[64:128, ds(s0, TC)]
                    fi_lo = fis[0:64, ds(s0, TC)]
                    fi_hi = fis[64:128, ds(s0, TC)]
                    t1 = ropet.tile([64, TC], F32, tag="t1")
                    t2 = ropet.tile([64, TC], F32, tag="t2")
                    nc.vector.tensor_mul(t1[:], ps[0:64, :], fr_lo)
                    nc.vector.tensor_mul(t2[:], ps[64:128, :], fi_hi)
                    nc.vector.tensor_sub(o_lo, t1[:], t2[:])
                    t3 = ropet.tile([64, TC], F32, tag="t3")
                    t4 = ropet.tile([64, TC], F32, tag="t4")
                    nc.vector.tensor_mul(t3[:], ps[64:128, :], fr_hi)
                    nc.vector.tensor_mul(t4[:], ps[0:64, :], fi_lo)
                    nc.vector.tensor_add(o_hi, t3[:], t4[:])
                    if f_lo is not None:
                        nc.vector.tensor_sub(f_lo, t1[:], t2[:])
                        nc.vector.tensor_add(f_hi, t3[:], t4[:])

                def proj_pass(do_q, do_kv, wq_sb, wk_sb, wv_sb):
                    from contextlib import ExitStack
                    ctx = ExitStack()
                    pjps = ctx.enter_context(
                        tc.tile_pool(name="pjps", bufs=1, space="PSUM"))
                    kvst = ctx.enter_context(tc.tile_pool(name="kvst", bufs=2))
                    vstage = ctx.enter_context(
                        tc.tile_pool(name="vstage", bufs=2))
                    pstage = ctx.enter_context(
                        tc.tile_pool(name="pstage", bufs=2))
                    tps = None
                    if do_kv:
                        tps = ctx.enter_context(
                            tc.tile_pool(name="tps", bufs=2, space="PSUM"))
                    for cp in range(NTC // 2):
                        cols2 = ds(cp * 2 * TC, 2 * TC)
                        qps = kps = vps = None
                        if do_q:
                            qps = [[pjps.tile([128, TC], F32,
                                              tag=f"qps{h}_{half}",
                                              name=f"qps{h}_{half}")
                                    for half in range(2)]
                                   for h in range(HPC)]
                        if do_kv:
                            kps = [pjps.tile([128, TC], F32, tag=f"kps{hf}",
                                             name=f"kps{hf}")
                                   for hf in range(2)]
                            vps = [pjps.tile([128, TC], F32, tag=f"vps{hf}",
                                             name=f"vps{hf}")
                                   for hf in range(2)]
                        for dt in range(ND):
                            xt_t = xinp.tile([128, 2 * TC], QKDT, tag="xt")
                            eng = nc.sync if dt % 2 == 0 else nc.gpsimd
                            eng.dma_start(
                                xt_t[:], xT[dt * 128:(dt + 1) * 128, cols2])
                            if do_kv and qk_f32:
                                xv_t = xinp.tile([128, 2 * TC], BF16,
                                                 tag="xvt")
                                eng2 = nc.gpsimd if dt % 2 == 0 else nc.sync
                                eng2.dma_start(
                                    xv_t[:],
                                    xTv[dt * 128:(dt + 1) * 128, cols2])
                            else:
                                xv_t = xt_t
                            st, sp = dt == 0, dt == ND - 1
                            if do_q:
                                for h in range(HPC):
                                    w_ap = wq_sb[:, ds(dt * HPC * HD
                                                       + h * HD, HD)]
                                    for hf in range(2):
                                        nc.tensor.matmul(
                                            qps[h][hf][:], w_ap,
                                            xt_t[:, ds(hf * TC, TC)],
                                            start=st, stop=sp)
                            if do_kv:
                                for hf in range(2):
                                    nc.tensor.matmul(
                                        kps[hf][:],
                                        wk_sb[:, ds(dt * HD, HD)],
                                        xt_t[:, ds(hf * TC, TC)],
                                        start=st, stop=sp)
                                for hf in range(2):
                                    nc.tensor.matmul(
                                        vps[hf][:],
                                        wv_sb[:, ds(dt * HD, HD)],
                                        xv_t[:, ds(hf * TC, TC)],
                                        start=st, stop=sp)
                        for hf in range(2):
                            c = cp * 2 + hf
                            cols = ds(c * TC, TC)
                            s0 = (c * TC) % S
                            if do_q:
                                for h in range(HPC):
                                    qst = pstage.tile([128, TC], F32,
                                                      tag="qst",
                                                      name="qst", bufs=2)
                                    nc.scalar.copy(qst[:], qps[h][hf][:])
                                    rope(qst, s0,
                                         qts[h][0:64, cols],
                                         qts[h][64:128, cols])
                            if do_kv:
                                vf_c = kvst.tile([128, TC], F32, tag="vfc")
                                kst = pstage.tile([128, TC], F32, tag="kst")
                                nc.scalar.copy(kst[:], kps[hf][:])
                                if qk_f32:
                                    rope(kst, s0, ktt[0:64, cols],
                                         ktt[64:128, cols])
                                    kf_c = None
                                else:
                                    kf_c = kvst.tile([128, TC], F32,
                                                     tag="kfc")
                                    rope(kst, s0, ktt[0:64, cols],
                                         ktt[64:128, cols],
                                         kf_c[0:64, :], kf_c[64:128, :])
                                nc.vector.tensor_copy(vf_c[:], vps[hf][:])
                                for t4 in range(TC // 128):
                                    tca = c * TC + t4 * 128
                                    lc = ds(t4 * 128, 128)
                                    kp = tps.tile([128, HD], F32, tag="tp")
                                    if qk_f32:
                                        nc.tensor.transpose(
                                            kp[:], ktt[:, ds(tca, 128)],
                                            permM[:])
                                    else:
                                        nc.tensor.transpose(
                                            kp[:], kf_c[:, lc], permM[:])
                                    ks = vstage.tile([128, HD], F32,
                                                     tag="ks")
                                    nc.vector.tensor_copy(ks[:], kp[:])
                                    nc.sync.dma_start(
                                        xk_d[tca:tca + 128, :], ks[:])
                                    vp = tps.tile([128, HD], F32, tag="tp")
                                    nc.tensor.transpose(vp[:], vf_c[:, lc],
                                                        identM[:])
                                    vs = vstage.tile([128, HD], F32,
                                                     tag="vs")
                                    nc.vector.tensor_copy(vs[:], vp[:])
                                    nc.sync.dma_start(
                                        xv_d[tca:tca + 128, :], vs[:])
                                    nc.scalar.copy(vtok[:, ds(tca, 128)],
                                                   vp[:])
                    ctx.close()

                with tc.tile_pool(name="wqp", bufs=1) as wqp:
                    wq_sb = wqp.tile([128, ND * HPC * HD], QKDT, tag="wq")
                    for dt in range(ND):
                        nc.sync.dma_start(
                            wq_sb[:, ds(dt * HPC * HD, HPC * HD)],
                            wqT[dt * 128:(dt + 1) * 128, :])
                    proj_pass(True, False, wq_sb, None, None)
                with tc.tile_pool(name="wkvp", bufs=1) as wkvp:
                    wk_sb = wkvp.tile([128, ND * HD], QKDT, tag="wk")
                    wv_sb = wkvp.tile([128, ND * HD], BF16, tag="wv")
                    for dt in range(ND):
                        nc.sync.dma_start(wk_sb[:, ds(dt * HD, HD)],
                                          wkT[dt * 128:(dt + 1) * 128, :])
                        nc.sync.dma_start(wv_sb[:, ds(dt * HD, HD)],
                                          wvT[dt * 128:(dt + 1) * 128, :])
                    proj_pass(False, True, None, wk_sb, wv_sb)

            # ================= Phase C: attention =================
            with (
                tc.tile_pool(name="sps", bufs=5, space="PSUM") as sps,
                tc.tile_pool(name="ptps", bufs=2, space="PSUM") as ptps,
                tc.tile_pool(name="atps", bufs=1, space="PSUM") as atps,
                tc.tile_pool(name="expp", bufs=4) as expp,
                tc.tile_pool(name="ptsb", bufs=3) as ptsb,
                tc.tile_pool(name="stat", bufs=8) as stat,
                tc.tile_pool(name="attsb", bufs=2) as attsb,
                tc.tile_pool(name="maskp", bufs=1) as maskp,
            ):
                masks = None
                if mixed_list:
                    masks = maskp.tile([128, len(mixed_list) * 128], MDT,
                                       tag="masks")
                    for i in range(len(mixed_list)):
                        nc.sync.dma_start(masks[:, ds(i * 128, 128)],
                                          maskb[i])
                for h in range(HPC):
                    qth = qts[h]
                    att_sbs = [attsb.tile([HD, S], BF16, tag="att",
                                          name=f"attsb{h}_{b}")
                               for b in range(B)]
                    for qb in range(NQB):
                        jl = int(jlim[qb])
                        if jl == 0:
                            for b in range(B):
                                nc.vector.memset(
                                    att_sbs[b][:, ds(qb * QB, QB)], 0.0)
                            continue
                        for b in range(B):
                            att_sb = att_sbs[b]
                            pt_sb = ptsb.tile([128, 4 * jl, QB], BF16,
                                              tag="pt")
                            for qtr in range(4):
                                qt = 4 * qb + qtr
                                qcols = ds(b * S + qt * 128, 128)
                                exp_sb = expp.tile([128, jl * KC], BF16,
                                                   tag="exp")
                                denom = stat.tile([128, 1], F32, tag="dn")
                                nasum = 0

                                def schunk(j):
                                    """compute S chunk j, return (sch, wj)."""
                                    subs = [int(blocks[qt, 4 * j + s])
                                            for s in range(4)]
                                    w = 4
                                    while w > 0 and subs[w - 1] == 1:
                                        w -= 1
                                    wj = w * KT
                                    sch = sps.tile([128, KC], F32, tag="s",
                                                   name="sch")
                                    if wj:
                                        nc.tensor.matmul(
                                            sch[:, 0:wj], qth[:, qcols],
                                            ktt[:, ds(b * S + j * KC, wj)],
                                            start=True, stop=True)
                                        for s2 in range(w):
                                            if subs[s2] == 1:
                                                nc.vector.tensor_scalar_add(
                                                    sch[:, ds(s2 * KT, KT)],
                                                    sch[:, ds(s2 * KT, KT)],
                                                    -1e9)
                                            elif subs[s2] == 2:
                                                mi = mixed_idx[(qt, 4 * j + s2)]
                                                nc.vector.tensor_add(
                                                    sch[:, ds(s2 * KT, KT)],
                                                    sch[:, ds(s2 * KT, KT)],
                                                    masks[:, ds(mi * 128, 128)])
                                    return sch, wj

                                def expchunk(j, sch, wj, bias):
                                    """exp + accumulate denom; free sch."""
                                    nonlocal nasum
                                    if wj:
                                        asum = stat.tile([128, 1], F32,
                                                         tag="as", name="as")
                                        nc.scalar.activation(
                                            exp_sb[:, ds(j * KC, wj)],
                                            sch[:, 0:wj],
                                            mybir.ActivationFunctionType.Exp,
                                            bias=bias, scale=1.0,
                                            accum_out=asum[:])
                                        if nasum == 0:
                                            nc.vector.tensor_copy(denom[:],
                                                                  asum[:])
                                        else:
                                            nc.vector.tensor_add(
                                                denom[:], denom[:], asum[:])
                                        nasum += 1
                                    if wj < KC:
                                        nc.vector.memset(
                                            exp_sb[:, ds(j * KC + wj,
                                                         KC - wj)], 0.0)

                                if exp_shift is not None:
                                    # no-max fast path: constant exp shift
                                    for j in range(jl):
                                        sch, wj = schunk(j)
                                        expchunk(j, sch, wj, shiftb[:])
                                    empty = nasum == 0
                                else:
                                    schunks = []
                                    runmax = None
                                    for j in range(jl):
                                        sch, wj = schunk(j)
                                        if wj:
                                            mx = stat.tile([128, 1], F32,
                                                           tag="mx",
                                                           name="mx")
                                            nc.vector.reduce_max(
                                                mx[:], sch[:, 0:wj],
                                                axis=mybir.AxisListType.X)
                                            if runmax is None:
                                                runmax = mx
                                            else:
                                                nc.vector.tensor_max(
                                                    runmax[:], runmax[:],
                                                    mx[:])
                                        schunks.append((sch, wj))
                                    empty = runmax is None
                                    if not empty:
                                        negmax = stat.tile([128, 1], F32,
                                                           tag="ng")
                                        nc.vector.tensor_scalar_mul(
                                            negmax[:], runmax[:], -1.0)
                                        for j, (sch, wj) in enumerate(schunks):
                                            expchunk(j, sch, wj, negmax[:])
                                    else:
                                        nc.vector.memset(exp_sb[:], 0.0)

                                diag = stat.tile([128, 128], BF16, tag="dg")
                                if empty:
                                    nc.vector.memset(exp_sb[:], 0.0)
                                    nc.vector.memset(diag[:], 0.0)
                                else:
                                    recip = stat.tile([128, 1], F32, tag="rc")
                                    nc.vector.reciprocal(recip[:], denom[:])
                                    nc.vector.tensor_scalar(
                                        diag[:], identMb[:], recip[:], None,
                                        op0=mybir.AluOpType.mult)
                                for j in range(jl):
                                    ptp = ptps.tile([128, 4, KT], F32,
                                                    tag="ptp")
                                    for s2 in range(4):
                                        nc.tensor.matmul(
                                            ptp[:, s2, :],
                                            exp_sb[:, ds(j * KC + s2 * KT,
                                                         KT)],
                                            diag[:], start=True, stop=True)
                                    dst = pt_sb[:, 4 * j:4 * j + 4,
                                                qtr * 128:(qtr + 1) * 128]
                                    if j % 2 == 0:
                                        nc.vector.tensor_copy(dst, ptp[:])
                                    else:
                                        nc.scalar.copy(dst, ptp[:])
                            att_ps = atps.tile([HD, QB], F32, tag="attps")
                            for k_i in range(4 * jl):
                                nc.tensor.matmul(
                                    att_ps[:],
                                    vtok[:, ds(b * S + k_i * KT, KT)],
                                    pt_sb[:, k_i, :],
                                    start=(k_i == 0), stop=(k_i == 4 * jl - 1))
                            nc.vector.tensor_copy(att_sb[:, ds(qb * QB, QB)],
                                                  att_ps[:])
                    for b in range(B):
                        for dest in range(N_CORES):
                            nc.sync.dma_start(
                                a2a_in[h][dest, :, b, :],
                                att_sbs[b][:, ds(dest * SC, SC)])
                    nc.gpsimd.collective_compute(
                        "AllToAll", mybir.AluOpType.bypass,
                        replica_groups=[list(range(N_CORES))],
                        ins=[a2a_in[h].opt()], outs=[a2a_out[h].opt()])

            qkv_ctx.__exit__(None, None, None)

            # ================= Phase D: output projection =================
            NE = H * HD // 128   # 32 e tiles
            NDC = D // 512       # 8 d chunks
            with (
                tc.tile_pool(name="attall", bufs=1) as attall,
                tc.tile_pool(name="wop", bufs=2) as wop,
                tc.tile_pool(name="ops", bufs=4, space="PSUM") as ops,
                tc.tile_pool(name="ostage", bufs=3) as ostage,
            ):
                attn_all = attall.tile([128, NE * TOKL], BF16, tag="attn_all")
                for src in range(N_CORES):
                    for h in range(HPC):
                        et = src * HPC + h
                        nc.sync.dma_start(attn_all[:, ds(et * TOKL, TOKL)],
                                          a2a_out[h][src].opt())
                for dc in range(NDC):
                    wo_sb = wop.tile([128, NE * 512], BF16, tag="wo")
                    for et in range(NE):
                        eng = nc.sync if et % 2 == 0 else nc.gpsimd
                        eng.dma_start(
                            wo_sb[:, ds(et * 512, 512)],
                            woT[et * 128:(et + 1) * 128,
                                dc * 512:(dc + 1) * 512])
                    for tt in range(TOKL // 128):
                        ps = ops.tile([128, 512], F32, tag="ops")
                        for et in range(NE):
                            nc.tensor.matmul(
                                ps[:],
                                attn_all[:, ds(et * TOKL + tt * 128, 128)],
                                wo_sb[:, ds(et * 512, 512)],
                                start=(et == 0), stop=(et == NE - 1))
                        ost = ostage.tile([128, 512], F32, tag="ost")
                        nc.vector.tensor_copy(ost[:], ps[:])
                        nc.sync.dma_start(
                            out_d[tt * 128:(tt + 1) * 128,
                                  dc * 512:(dc + 1) * 512], ost[:])

    nc.compile()
    return nc


# ---------------------------------------------------------------- kernel()

def _prep(inputs):
    x = np.asarray(inputs["x"], np.float32)
    wq = np.asarray(inputs["wq"], np.float32)
    wk = np.asarray(inputs["wk"], np.float32)
    wv = np.asarray(inputs["wv"], np.float32)
    wo = np.asarray(inputs["wo"], np.float32)
    fr = np.asarray(inputs["fr"], np.float32)
    fi = np.asarray(inputs["fi"], np.float32)
    mask = np.asarray(inputs["mask"], np.float32)
    indexes = np.asarray(inputs["indexes"]).ravel()
    cache_indexes = np.asarray(inputs["cache_indexes"]).ravel()

    if not (np.array_equal(indexes, np.arange(S)) and
            np.array_equal(cache_indexes, np.arange(S))):
        raise NotImplementedError("only identity cache indexes supported")

    perm = _deinterleave_perm()
    wq_p = wq.reshape(H, HD, D)[:, perm, :] / np.sqrt(np.float32(HD))
    wk_p = wk.reshape(KV, HD, D)[:, perm, :]

    blocks, jlim = _classify_mask(mask[0, 0])
    n_mixed = int((blocks == 2).sum())
    mask_f32 = n_mixed <= 64
    smax = _estimate_score_max(x, wq_p.reshape(H * HD, D),
                               wk_p.reshape(KV * HD, D), fr, fi)
    qk_f32 = bool(smax > 20.0)
    # no-max fast path: constant exp shift, if scores are provably tame.
    exp_shift = None
    rmm = _estimate_rowmax_min(x, wq_p.reshape(H * HD, D),
                               wk_p.reshape(KV * HD, D), fr, fi, mask[0, 0])
    cshift = max(0.0, smax - 10.0)
    no_rows_fully_masked = bool((mask[0, 0].max(axis=1) > -1e8).all())
    if (not qk_f32 and smax < 40.0 and no_rows_fully_masked
            and rmm - cshift > -25.0):
        exp_shift = float(cshift)
    import os
    force = os.environ.get("KERNEL_QK_MODE", "")
    if force == "bf16":
        qk_f32 = False
    elif force == "f32":
        qk_f32 = True
        exp_shift = None
    if os.environ.get("KERNEL_USE_MAX", "") == "1":
        exp_shift = None
    print(f"[kernel] smax_est={smax:.2f} rowmaxmin_est={rmm:.2f} "
          f"qk_f32={qk_f32} exp_shift={exp_shift}")

    QNP = np.float32 if qk_f32 else BF16_NP
    MNP = np.float32 if mask_f32 else BF16_NP

    xTf = np.ascontiguousarray(x.reshape(T, D).T)
    mixed_list = [(qt, kt) for qt in range(16) for kt in range(16)
                  if blocks[qt, kt] == 2]
    m00 = mask[0, 0]
    maskb_s = None
    if mixed_list:
        maskb_s = np.stack([
            m00[qt * 128:(qt + 1) * 128, kt * 128:(kt + 1) * 128]
            for qt, kt in mixed_list]).astype(MNP)

    shared = dict(
        xT=xTf.astype(QNP),
        woT=np.ascontiguousarray(wo.T).astype(BF16_NP),
        frT=np.ascontiguousarray(fr.T),
        fiT=np.ascontiguousarray(fi.T),
    )
    if qk_f32:
        shared["xTv"] = xTf.astype(BF16_NP)
    if maskb_s is not None:
        shared["maskb"] = maskb_s

    in_maps = []
    for c in range(N_CORES):
        wq_c = wq_p[c * HPC:(c + 1) * HPC].reshape(HPC * HD, D)
        wk_c = wk_p[c]
        wv_c = wv.reshape(KV, HD, D)[c]
        m = dict(shared)
        m["wqT"] = np.ascontiguousarray(wq_c.T).astype(QNP)
        m["wkT"] = np.ascontiguousarray(wk_c.T).astype(QNP)
        m["wvT"] = np.ascontiguousarray(wv_c.T).astype(BF16_NP)
        in_maps.append(m)
    return in_maps, qk_f32, blocks, jlim, mask_f32, exp_shift


_PROGRAM_CACHE = {}


def run(inputs, trace=False):
    in_maps, qk_f32, blocks, jlim, mask_f32, exp_shift = _prep(inputs)
    key = (qk_f32, blocks.tobytes(), jlim.tobytes(), mask_f32, exp_shift)
    if key not in _PROGRAM_CACHE:
        _PROGRAM_CACHE[key] = build_program(qk_f32, blocks, jlim, mask_f32,
                                            exp_shift)
    nc = _PROGRAM_CACHE[key]
    res = run_bass_kernel_spmd(nc, in_maps, core_ids=list(range(N_CORES)),
                               trace=trace)
    out = np.empty((B, S, D), np.float32)
    xk = np.empty((B, S, KV, HD), np.float32)
    xv = np.empty((B, S, KV, HD), np.float32)
    for c in range(N_CORES):
        r = res.results[c]
        out[:, c * SC:(c + 1) * SC, :] = r["out"].reshape(B, SC, D)
        xk[:, :, c, :] = r["xk"].reshape(B, S, HD)
        xv[:, :, c, :] = r["xv"].reshape(B, S, HD)
    return (out, xk, xv), res


def kernel(**inputs):
    (out, xk, xv), _ = run(inputs, trace=False)
    return out, xk, xv


# revision 18
# speedup vs baseline: 1.1089x; 1.1089x over previous
"""Distributed GQA attention kernel for Trainium2 (8 NeuronCores).

Problem: nn_Attention_69595650065097
  B=2, S=2048, D=4096, H=32 q-heads, KV=8 kv-heads, HD=128.
  reference returns (out[B,S,D], xk[B,S,KV,HD], xv[B,S,KV,HD]).

Sharding: tensor-parallel over heads. Each of the 8 cores owns 4 q-heads and
1 kv-head, computes attention for them over the full sequence, then an
AllToAll redistributes attention outputs so each core owns a 1/8 slice of the
sequence with the full hidden dim for the output projection (full wo on every
core, streamed from HBM).

Numerics: all matmuls run in bf16 (4x faster on the PE than f32) unless a
host-side estimate of the score magnitude says bf16 score error would corrupt
the softmax, in which case the Q/K path (projections + scores) runs in f32.

Host-side layout tricks folded into the shards (free):
  - RoPE de-interleave permutation baked into wq/wk rows, so on-device rope is
    contiguous-half form; a permutation matrix in the xk transpose undoes it.
  - 1/sqrt(HD) folded into wq.
  - x, fr, fi, wo pre-transposed to the layouts the device wants.
"""

import numpy as np
import ml_dtypes

import concourse.bass as bass
import concourse.mybir as mybir
import concourse.tile as tile
from concourse import bacc
from concourse.bass import ds
from concourse.bass_utils import run_bass_kernel_spmd

B, S, D, H, KV, HD = 2, 2048, 4096, 32, 8, 128
N_CORES = 8
HPC = H // N_CORES            # q heads per core = 4
T = B * S                     # tokens = 4096
TC = 512                      # token chunk for projections
NTC = T // TC                 # 8
ND = D // 128                 # 32 d-tiles
SC = S // N_CORES             # 256: seq slice per core after A2A
TOKL = B * SC                 # 512 local tokens for wo
QB = 512                      # q block in attention
NQB = S // QB                 # 4
KT = 128                      # k tile
KC = 512                      # k chunk (4 k tiles)

F32 = mybir.dt.float32
BF16 = mybir.dt.bfloat16
BF16_NP = ml_dtypes.bfloat16


# ---------------------------------------------------------------- host prep

def _deinterleave_perm():
    return np.concatenate([np.arange(0, HD, 2), np.arange(1, HD, 2)])


def _classify_mask(mask):
    """mask: [S, S] f32. 128x128 blocks: 0=zero, 1=neginf, 2=mixed.
    Returns (blocks[16,16] int8, jlim[NQB]: #512-chunks to compute/q-block)."""
    nq = S // 128
    m = mask.reshape(nq, 128, nq, 128)
    bmax = m.max(axis=(1, 3))
    bmin = m.min(axis=(1, 3))
    blocks = np.full((nq, nq), 2, np.int8)
    blocks[(bmax == 0) & (bmin == 0)] = 0
    blocks[bmax <= -1e8] = 1
    jlim = np.zeros(NQB, np.int64)
    for qb in range(NQB):
        lim = 0
        for qt in range(4 * qb, 4 * qb + 4):
            nonneg = np.nonzero(blocks[qt] != 1)[0]
            if len(nonneg):
                lim = max(lim, int(nonneg[-1]) // 4 + 1)
        jlim[qb] = lim
    return blocks, jlim


def _estimate_score_max(x, wq_scaled_perm, wk_perm, fr, fi):
    """Cheap host-side estimate of max |score| using sampled tokens."""
    rng = np.random.default_rng(1234)
    qs = np.sort(rng.choice(S, 32, replace=False))
    ks = np.sort(rng.choice(S, 256, replace=False))

    def proj_rope(tok_idx, w, nh):
        v = x[0, tok_idx].astype(np.float64) @ w.astype(np.float64).T
        v = v.reshape(len(tok_idx), nh, HD)
        lo, hi = v[..., :64], v[..., 64:]
        f_r = fr[tok_idx][:, None, :]
        f_i = fi[tok_idx][:, None, :]
        return np.concatenate([lo * f_r - hi * f_i, hi * f_r + lo * f_i], -1)

    q = proj_rope(qs, wq_scaled_perm, H)
    k = proj_rope(ks, wk_perm, KV)
    kk = np.repeat(k, H // KV, axis=1)
    s = np.einsum("qhd,khd->hqk", q, kk)
    return float(np.abs(s).max())


def _estimate_rowmax_min(x, wq_scaled_perm, wk_perm, fr, fi, mask):
    """Estimate min over rows of the row-max score (sampled, mask-aware)."""
    rng = np.random.default_rng(99)
    qs = np.sort(rng.choice(S, 48, replace=False))
    ks = np.sort(np.unique(np.concatenate([qs, rng.choice(S, 192,
                                                          replace=False)])))

    def proj_rope(tok_idx, w, nh, b):
        v = x[b, tok_idx].astype(np.float64) @ w.astype(np.float64).T
        v = v.reshape(len(tok_idx), nh, HD)
        lo, hi = v[..., :64], v[..., 64:]
        f_r = fr[tok_idx][:, None, :]
        f_i = fi[tok_idx][:, None, :]
        return np.concatenate([lo * f_r - hi * f_i, hi * f_r + lo * f_i], -1)

    worst = np.inf
    for b in range(B):
        q = proj_rope(qs, wq_scaled_perm, H, b)
        k = proj_rope(ks, wk_perm, KV, b)
        kk = np.repeat(k, H // KV, axis=1)
        sc = np.einsum("qhd,khd->hqk", q, kk)
        sc = sc + mask[np.ix_(qs, ks)][None]
        rm = sc.max(axis=2)          # [h, q] row maxes over sampled k
        valid = (mask[np.ix_(qs, ks)].max(axis=1) > -1e8)
        if valid.any():
            worst = min(worst, float(rm[:, valid].min()))
    return worst


# ---------------------------------------------------------------- device IR

def build_program(qk_f32, blocks, jlim, mask_f32, exp_shift):
    """Build the SPMD Bass program (identical on all 8 cores)."""
    QKDT = F32 if qk_f32 else BF16
    MDT = F32 if mask_f32 else BF16

    nc = bacc.Bacc("TRN2", target_bir_lowering=False, debug=False,
                   num_devices=N_CORES)

    # ---- I/O ----
    xT = nc.dram_tensor("xT", [D, T], QKDT, kind="ExternalInput")
    xTv = None
    if qk_f32:
        xTv = nc.dram_tensor("xTv", [D, T], BF16, kind="ExternalInput")
    wqT = nc.dram_tensor("wqT", [D, HPC * HD], QKDT, kind="ExternalInput")
    wkT = nc.dram_tensor("wkT", [D, HD], QKDT, kind="ExternalInput")
    wvT = nc.dram_tensor("wvT", [D, HD], BF16, kind="ExternalInput")
    woT = nc.dram_tensor("woT", [H * HD, D], BF16, kind="ExternalInput")
    frT = nc.dram_tensor("frT", [64, S], F32, kind="ExternalInput")
    fiT = nc.dram_tensor("fiT", [64, S], F32, kind="ExternalInput")
    mixed_list = [(qt, kt) for qt in range(16) for kt in range(16)
                  if blocks[qt, kt] == 2]
    maskb = None
    if mixed_list:
        maskb = nc.dram_tensor("maskb", [len(mixed_list), 128, 128], MDT,
                               kind="ExternalInput")
    mixed_idx = {qk: i for i, qk in enumerate(mixed_list)}

    out_d = nc.dram_tensor("out", [TOKL, D], F32, kind="ExternalOutput")
    xk_d = nc.dram_tensor("xk", [T, HD], F32, kind="ExternalOutput")
    xv_d = nc.dram_tensor("xv", [T, HD], F32, kind="ExternalOutput")

    perm = _deinterleave_perm()
    perm_mat = np.zeros((HD, HD), np.float32)
    perm_mat[np.arange(HD), perm] = 1.0
    permM_d = nc.inline_tensor(perm_mat, "permM")
    ident_d = nc.inline_tensor(np.eye(HD, dtype=np.float32), "identM")
    identb_d = nc.inline_tensor(np.eye(HD, dtype=BF16_NP), "identMb")

    with tile.TileContext(nc) as tc:
        with (
            tc.tile_pool(name="const", bufs=1) as constp,
            tc.tile_pool(name="dram", bufs=1, space="DRAM") as dram,
        ):
            # ---- constants ----
            frs = constp.tile([128, S], F32, tag="frs")
            fis = constp.tile([128, S], F32, tag="fis")
            nc.sync.dma_start(frs[0:64, :], frT[:])
            nc.sync.dma_start(frs[64:128, :], frT[:])
            nc.sync.dma_start(fis[0:64, :], fiT[:])
            nc.sync.dma_start(fis[64:128, :], fiT[:])
            permM = constp.tile([HD, HD], F32, tag="permM")
            nc.sync.dma_start(permM[:], permM_d[:])
            identM = constp.tile([HD, HD], F32, tag="identM")
            nc.sync.dma_start(identM[:], ident_d[:])
            identMb = constp.tile([HD, HD], BF16, tag="identMb")
            nc.sync.dma_start(identMb[:], identb_d[:])
            shiftb = None
            if exp_shift is not None:
                shiftb = constp.tile([128, 1], F32, tag="shiftb")
                nc.vector.memset(shiftb[:], -float(exp_shift))


            # ---- A2A bounce buffers ----
            a2a_in = [dram.tile([N_CORES, HD, B, SC], BF16,
                                name=f"a2ain{h}") for h in range(HPC)]
            a2a_out = [dram.tile([N_CORES, HD, B, SC], BF16,
                                 name=f"a2aout{h}") for h in range(HPC)]

            # ---- residents for phases B+C (freed before phase D) ----
            qkv_ctx = tc.tile_pool(name="qkv", bufs=1)
            qkvp = qkv_ctx.__enter__()
            qts = [qkvp.tile([HD, T], QKDT, tag=f"qt{h}", name=f"qt{h}")
                   for h in range(HPC)]
            kf = (qkvp.tile([HD, T], F32, tag="kf", name="kf")
                  if qk_f32 else None)
            ktt = (kf if qk_f32 else
                   qkvp.tile([HD, T], BF16, tag="ktt", name="ktt"))
            vtok = qkvp.tile([128, T], BF16, tag="vtok")  # V token-major

            # ================= Phase B: projections + rope =================
            # In f32 QK mode SBUF cannot hold wq(f32) + wk + wv at once, so
            # run two passes (Q-only, then K+V), streaming xT twice.
            with (
                tc.tile_pool(name="xin", bufs=(3 if qk_f32 else 6)) as xinp,
                tc.tile_pool(name="ropet", bufs=1) as ropet,
                            ):

                def rope(ps, s0, o_lo, o_hi, f_lo=None, f_hi=None):
                    fr_lo = frs[0:64, ds(s0, TC)]
                    fr_hi = frs[64:128, ds(s0, TC)]
                    fi_lo = fis[0:64, ds(s0, TC)]
                    fi_hi = fis[64:128, ds(s0, TC)]
                    t1 = ropet.tile([64, TC], F32, tag="t1")
                    t2 = ropet.tile([64, TC], F32, tag="t2")
                    nc.vector.tensor_mul(t1[:], ps[0:64, :], fr_lo)
                    nc.vector.tensor_mul(t2[:], ps[64:128, :], fi_hi)
                    nc.vector.tensor_sub(o_lo, t1[:], t2[:])
                    t3 = ropet.tile([64, TC], F32, tag="t3")
                    t4 = ropet.tile([64, TC], F32, tag="t4")
                    nc.vector.tensor_mul(t3[:], ps[64:128, :], fr_hi)
                    nc.vector.tensor_mul(t4[:], ps[0:64, :], fi_lo)
                    nc.vector.tensor_add(o_hi, t3[:], t4[:])
                    if f_lo is not None:
                        nc.vector.tensor_sub(f_lo, t1[:], t2[:])
                        nc.vector.tensor_add(f_hi, t3[:], t4[:])

                def proj_pass(do_q, do_kv, wq_sb, wk_sb, wv_sb):
                    from contextlib import ExitStack
                    ctx = ExitStack()
                    pjps = ctx.enter_context(
                        tc.tile_pool(name="pjps", bufs=1, space="PSUM"))
                    kvst = ctx.enter_context(tc.tile_pool(name="kvst", bufs=2))
                    vstage = ctx.enter_context(
                        tc.tile_pool(name="vstage", bufs=2))
                    pstage = ctx.enter_context(
                        tc.tile_pool(name="pstage", bufs=2))
                    tps = None
                    if do_kv:
                        tps = ctx.enter_context(
                            tc.tile_pool(name="tps", bufs=2, space="PSUM"))
                    for c in range(NTC):
                        cols = ds(c * TC, TC)
                        qps = kps = vps = None
                        if do_q:
                            qps = [pjps.tile([128, TC], F32, tag=f"qps{h}",
                                             name=f"qps{h}")
                                   for h in range(HPC)]
                        if do_kv:
                            kps = pjps.tile([128, TC], F32, tag="kps")
                            vps = pjps.tile([128, TC], F32, tag="vps")
                        for dt in range(ND):
                            xt_t = xinp.tile([128, TC], QKDT, tag="xt")
                            eng = nc.sync if dt % 2 == 0 else nc.gpsimd
                            eng.dma_start(
                                xt_t[:], xT[dt * 128:(dt + 1) * 128, cols])
                            if do_kv and qk_f32:
                                xv_t = xinp.tile([128, TC], BF16, tag="xvt")
                                eng2 = nc.gpsimd if dt % 2 == 0 else nc.sync
                                eng2.dma_start(
                                    xv_t[:],
                                    xTv[dt * 128:(dt + 1) * 128, cols])
                            else:
                                xv_t = xt_t
                            st, sp = dt == 0, dt == ND - 1
                            if do_q:
                                for h in range(HPC):
                                    nc.tensor.matmul(
                                        qps[h][:],
                                        wq_sb[:, ds(dt * HPC * HD
                                                    + h * HD, HD)],
                                        xt_t[:], start=st, stop=sp)
                            if do_kv:
                                nc.tensor.matmul(
                                    kps[:], wk_sb[:, ds(dt * HD, HD)],
                                    xt_t[:], start=st, stop=sp)
                                nc.tensor.matmul(
                                    vps[:], wv_sb[:, ds(dt * HD, HD)],
                                    xv_t[:], start=st, stop=sp)
                        s0 = (c * TC) % S
                        if do_q:
                            for h in range(HPC):
                                qst = pstage.tile([128, TC], F32, tag="qst",
                                                  name="qst", bufs=2)
                                nc.scalar.copy(qst[:], qps[h][:])
                                rope(qst, s0, qts[h][0:64, cols],
                                     qts[h][64:128, cols])
                        if do_kv:
                            vf_c = kvst.tile([128, TC], F32, tag="vfc")
                            kst = pstage.tile([128, TC], F32, tag="kst")
                            nc.scalar.copy(kst[:], kps[:])
                            if qk_f32:
                                rope(kst, s0, ktt[0:64, cols],
                                     ktt[64:128, cols])
                                kf_c = None
                            else:
                                kf_c = kvst.tile([128, TC], F32, tag="kfc")
                                rope(kst, s0, ktt[0:64, cols],
                                     ktt[64:128, cols],
                                     kf_c[0:64, :], kf_c[64:128, :])
                            nc.vector.tensor_copy(vf_c[:], vps[:])
                            for t4 in range(TC // 128):
                                tca = c * TC + t4 * 128
                                lc = ds(t4 * 128, 128)
                                kp = tps.tile([128, HD], F32, tag="tp")
                                if qk_f32:
                                    nc.tensor.transpose(
                                        kp[:], ktt[:, ds(tca, 128)],
                                        permM[:])
                                else:
                                    nc.tensor.transpose(
                                        kp[:], kf_c[:, lc], permM[:])
                                ks = vstage.tile([128, HD], F32, tag="ks")
                                nc.vector.tensor_copy(ks[:], kp[:])
                                nc.sync.dma_start(
                                    xk_d[tca:tca + 128, :], ks[:])
                                vp = tps.tile([128, HD], F32, tag="tp")
                                nc.tensor.transpose(vp[:], vf_c[:, lc],
                                                    identM[:])
                                vs = vstage.tile([128, HD], F32, tag="vs")
                                nc.vector.tensor_copy(vs[:], vp[:])
                                nc.sync.dma_start(
                                    xv_d[tca:tca + 128, :], vs[:])
                                nc.scalar.copy(vtok[:, ds(tca, 128)], vp[:])
                    ctx.close()

                if qk_f32:
                    with tc.tile_pool(name="wqp", bufs=1) as wqp:
                        wq_sb = wqp.tile([128, ND * HPC * HD], QKDT, tag="wq")
                        for dt in range(ND):
                            nc.sync.dma_start(
                                wq_sb[:, ds(dt * HPC * HD, HPC * HD)],
                                wqT[dt * 128:(dt + 1) * 128, :])
                        proj_pass(True, False, wq_sb, None, None)
                    with tc.tile_pool(name="wkvp", bufs=1) as wkvp:
                        wk_sb = wkvp.tile([128, ND * HD], QKDT, tag="wk")
                        wv_sb = wkvp.tile([128, ND * HD], BF16, tag="wv")
                        for dt in range(ND):
                            nc.sync.dma_start(wk_sb[:, ds(dt * HD, HD)],
                                              wkT[dt * 128:(dt + 1) * 128, :])
                            nc.sync.dma_start(wv_sb[:, ds(dt * HD, HD)],
                                              wvT[dt * 128:(dt + 1) * 128, :])
                        proj_pass(False, True, None, wk_sb, wv_sb)
                else:
                    with tc.tile_pool(name="wqp", bufs=1) as wqp:
                        wq_sb = wqp.tile([128, ND * HPC * HD], QKDT, tag="wq")
                        wk_sb = wqp.tile([128, ND * HD], QKDT, tag="wk")
                        wv_sb = wqp.tile([128, ND * HD], BF16, tag="wv")
                        for dt in range(ND):
                            nc.sync.dma_start(
                                wq_sb[:, ds(dt * HPC * HD, HPC * HD)],
                                wqT[dt * 128:(dt + 1) * 128, :])
                            nc.sync.dma_start(wk_sb[:, ds(dt * HD, HD)],
                                              wkT[dt * 128:(dt + 1) * 128, :])
                            nc.sync.dma_start(wv_sb[:, ds(dt * HD, HD)],
                                              wvT[dt * 128:(dt + 1) * 128, :])
                        proj_pass(True, True, wq_sb, wk_sb, wv_sb)

            # ================= Phase C: attention =================
            with (
                tc.tile_pool(name="sps", bufs=5, space="PSUM") as sps,
                tc.tile_pool(name="ptps", bufs=2, space="PSUM") as ptps,
                tc.tile_pool(name="atps", bufs=1, space="PSUM") as atps,
                tc.tile_pool(name="expp", bufs=4) as expp,
                tc.tile_pool(name="ptsb", bufs=3) as ptsb,
                tc.tile_pool(name="stat", bufs=8) as stat,
                tc.tile_pool(name="attsb", bufs=2) as attsb,
                tc.tile_pool(name="maskp", bufs=1) as maskp,
            ):
                masks = None
                if mixed_list:
                    masks = maskp.tile([128, len(mixed_list) * 128], MDT,
                                       tag="masks")
                    for i in range(len(mixed_list)):
                        nc.sync.dma_start(masks[:, ds(i * 128, 128)],
                                          maskb[i])
                for h in range(HPC):
                    qth = qts[h]
                    att_sbs = [attsb.tile([HD, S], BF16, tag="att",
                                          name=f"attsb{h}_{b}")
                               for b in range(B)]
                    for qb in range(NQB):
                        jl = int(jlim[qb])
                        if jl == 0:
                            for b in range(B):
                                nc.vector.memset(
                                    att_sbs[b][:, ds(qb * QB, QB)], 0.0)
                            continue
                        for b in range(B):
                            att_sb = att_sbs[b]
                            pt_sb = ptsb.tile([128, 4 * jl, QB], BF16,
                                              tag="pt")
                            for qtr in range(4):
                                qt = 4 * qb + qtr
                                qcols = ds(b * S + qt * 128, 128)
                                exp_sb = expp.tile([128, jl * KC], BF16,
                                                   tag="exp")
                                denom = stat.tile([128, 1], F32, tag="dn")
                                nasum = 0

                                def schunk(j):
                                    """compute S chunk j, return (sch, wj)."""
                                    subs = [int(blocks[qt, 4 * j + s])
                                            for s in range(4)]
                                    w = 4
                                    while w > 0 and subs[w - 1] == 1:
                                        w -= 1
                                    wj = w * KT
                                    sch = sps.tile([128, KC], F32, tag="s",
                                                   name="sch")
                                    if wj:
                                        nc.tensor.matmul(
                                            sch[:, 0:wj], qth[:, qcols],
                                            ktt[:, ds(b * S + j * KC, wj)],
                                            start=True, stop=True)
                                        for s2 in range(w):
                                            if subs[s2] == 1:
                                                nc.vector.tensor_scalar_add(
                                                    sch[:, ds(s2 * KT, KT)],
                                                    sch[:, ds(s2 * KT, KT)],
                                                    -1e9)
                                            elif subs[s2] == 2:
                                                mi = mixed_idx[(qt, 4 * j + s2)]
                                                nc.vector.tensor_add(
                                                    sch[:, ds(s2 * KT, KT)],
                                                    sch[:, ds(s2 * KT, KT)],
                                                    masks[:, ds(mi * 128, 128)])
                                    return sch, wj

                                def expchunk(j, sch, wj, bias):
                                    """exp + accumulate denom; free sch."""
                                    nonlocal nasum
                                    if wj:
                                        asum = stat.tile([128, 1], F32,
                                                         tag="as", name="as")
                                        nc.scalar.activation(
                                            exp_sb[:, ds(j * KC, wj)],
                                            sch[:, 0:wj],
                                            mybir.ActivationFunctionType.Exp,
                                            bias=bias, scale=1.0,
                                            accum_out=asum[:])
                                        if nasum == 0:
                                            nc.vector.tensor_copy(denom[:],
                                                                  asum[:])
                                        else:
                                            nc.vector.tensor_add(
                                                denom[:], denom[:], asum[:])
                                        nasum += 1
                                    if wj < KC:
                                        nc.vector.memset(
                                            exp_sb[:, ds(j * KC + wj,
                                                         KC - wj)], 0.0)

                                if exp_shift is not None:
                                    # no-max fast path: constant exp shift
                                    for j in range(jl):
                                        sch, wj = schunk(j)
                                        expchunk(j, sch, wj, shiftb[:])
                                    empty = nasum == 0
                                else:
                                    schunks = []
                                    runmax = None
                                    for j in range(jl):
                                        sch, wj = schunk(j)
                                        if wj:
                                            mx = stat.tile([128, 1], F32,
                                                           tag="mx",
                                                           name="mx")
                                            nc.vector.reduce_max(
                                                mx[:], sch[:, 0:wj],
                                                axis=mybir.AxisListType.X)
                                            if runmax is None:
                                                runmax = mx
                                            else:
                                                nc.vector.tensor_max(
                                                    runmax[:], runmax[:],
                                                    mx[:])
                                        schunks.append((sch, wj))
                                    empty = runmax is None
                                    if not empty:
                                        negmax = stat.tile([128, 1], F32,
                                                           tag="ng")
                                        nc.vector.tensor_scalar_mul(
                                            negmax[:], runmax[:], -1.0)
                                        for j, (sch, wj) in enumerate(schunks):
                                            expchunk(j, sch, wj, negmax[:])
                                    else:
                                        nc.vector.memset(exp_sb[:], 0.0)

                                diag = stat.tile([128, 128], BF16, tag="dg")
                                if empty:
                                    nc.vector.memset(exp_sb[:], 0.0)
                                    nc.vector.memset(diag[:], 0.0)
                                else:
                                    recip = stat.tile([128, 1], F32, tag="rc")
                                    nc.vector.reciprocal(recip[:], denom[:])
                                    nc.vector.tensor_scalar(
                                        diag[:], identMb[:], recip[:], None,
                                        op0=mybir.AluOpType.mult)
                                for j in range(jl):
                                    ptp = ptps.tile([128, 4, KT], F32,
                                                    tag="ptp")
                                    for s2 in range(4):
                                        nc.tensor.matmul(
                                            ptp[:, s2, :],
                                            exp_sb[:, ds(j * KC + s2 * KT,
                                                         KT)],
                                            diag[:], start=True, stop=True)
                                    dst = pt_sb[:, 4 * j:4 * j + 4,
                                                qtr * 128:(qtr + 1) * 128]
                                    if j % 2 == 0:
                                        nc.vector.tensor_copy(dst, ptp[:])
                                    else:
                                        nc.scalar.copy(dst, ptp[:])
                            att_ps = atps.tile([HD, QB], F32, tag="attps")
                            for k_i in range(4 * jl):
                                nc.tensor.matmul(
                                    att_ps[:],
                                    vtok[:, ds(b * S + k_i * KT, KT)],
                                    pt_sb[:, k_i, :],
                                    start=(k_i == 0), stop=(k_i == 4 * jl - 1))
                            nc.vector.tensor_copy(att_sb[:, ds(qb * QB, QB)],
                                                  att_ps[:])
                    for b in range(B):
                        for dest in range(N_CORES):
                            nc.sync.dma_start(
                                a2a_in[h][dest, :, b, :],
                                att_sbs[b][:, ds(dest * SC, SC)])
                    nc.gpsimd.collective_compute(
                        "AllToAll", mybir.AluOpType.bypass,
                        replica_groups=[list(range(N_CORES))],
                        ins=[a2a_in[h].opt()], outs=[a2a_out[h].opt()])

            qkv_ctx.__exit__(None, None, None)

            # ================= Phase D: output projection =================
            NE = H * HD // 128   # 32 e tiles
            NDC = D // 512       # 8 d chunks
            with (
                tc.tile_pool(name="attall", bufs=1) as attall,
                tc.tile_pool(name="wop", bufs=2) as wop,
                tc.tile_pool(name="ops", bufs=4, space="PSUM") as ops,
                tc.tile_pool(name="ostage", bufs=3) as ostage,
            ):
                attn_all = attall.tile([128, NE * TOKL], BF16, tag="attn_all")
                for src in range(N_CORES):
                    for h in range(HPC):
                        et = src * HPC + h
                        nc.sync.dma_start(attn_all[:, ds(et * TOKL, TOKL)],
                                          a2a_out[h][src].opt())
                for dc in range(NDC):
                    wo_sb = wop.tile([128, NE * 512], BF16, tag="wo")
                    for et in range(NE):
                        eng = nc.sync if et % 2 == 0 else nc.gpsimd
                        eng.dma_start(
                            wo_sb[:, ds(et * 512, 512)],
                            woT[et * 128:(et + 1) * 128,
                                dc * 512:(dc + 1) * 512])
                    for tt in range(TOKL // 128):
                        ps = ops.tile([128, 512], F32, tag="ops")
                        for et in range(NE):
                            nc.tensor.matmul(
                                ps[:],
                                attn_all[:, ds(et * TOKL + tt * 128, 128)],
                                wo_sb[:, ds(et * 512, 512)],
                                start=(et == 0), stop=(et == NE - 1))
                        ost = ostage.tile([128, 512], F32, tag="ost")
                        nc.vector.tensor_copy(ost[:], ps[:])
                        nc.sync.dma_start(
                            out_d[tt * 128:(tt + 1) * 128,
                                  dc * 512:(dc + 1) * 512], ost[:])

    nc.compile()
    return nc


# ---------------------------------------------------------------- kernel()

def _prep(inputs):
    x = np.asarray(inputs["x"], np.float32)
    wq = np.asarray(inputs["wq"], np.float32)
    wk = np.asarray(inputs["wk"], np.float32)
    wv = np.asarray(inputs["wv"], np.float32)
    wo = np.asarray(inputs["wo"], np.float32)
    fr = np.asarray(inputs["fr"], np.float32)
    fi = np.asarray(inputs["fi"], np.float32)
    mask = np.asarray(inputs["mask"], np.float32)
    indexes = np.asarray(inputs["indexes"]).ravel()
    cache_indexes = np.asarray(inputs["cache_indexes"]).ravel()

    if not (np.array_equal(indexes, np.arange(S)) and
            np.array_equal(cache_indexes, np.arange(S))):
        raise NotImplementedError("only identity cache indexes supported")

    perm = _deinterleave_perm()
    wq_p = wq.reshape(H, HD, D)[:, perm, :] / np.sqrt(np.float32(HD))
    wk_p = wk.reshape(KV, HD, D)[:, perm, :]

    blocks, jlim = _classify_mask(mask[0, 0])
    n_mixed = int((blocks == 2).sum())
    mask_f32 = n_mixed <= 64
    smax = _estimate_score_max(x, wq_p.reshape(H * HD, D),
                               wk_p.reshape(KV * HD, D), fr, fi)
    qk_f32 = bool(smax > 20.0)
    # no-max fast path: constant exp shift, if scores are provably tame.
    exp_shift = None
    rmm = _estimate_rowmax_min(x, wq_p.reshape(H * HD, D),
                               wk_p.reshape(KV * HD, D), fr, fi, mask[0, 0])
    cshift = max(0.0, smax - 10.0)
    no_rows_fully_masked = bool((mask[0, 0].max(axis=1) > -1e8).all())
    if (not qk_f32 and smax < 40.0 and no_rows_fully_masked
            and rmm - cshift > -25.0):
        exp_shift = float(cshift)
    import os
    force = os.environ.get("KERNEL_QK_MODE", "")
    if force == "bf16":
        qk_f32 = False
    elif force == "f32":
        qk_f32 = True
        exp_shift = None
    if os.environ.get("KERNEL_USE_MAX", "") == "1":
        exp_shift = None
    print(f"[kernel] smax_est={smax:.2f} rowmaxmin_est={rmm:.2f} "
          f"qk_f32={qk_f32} exp_shift={exp_shift}")

    QNP = np.float32 if qk_f32 else BF16_NP
    MNP = np.float32 if mask_f32 else BF16_NP

    xTf = np.ascontiguousarray(x.reshape(T, D).T)
    mixed_list = [(qt, kt) for qt in range(16) for kt in range(16)
                  if blocks[qt, kt] == 2]
    m00 = mask[0, 0]
    maskb_s = None
    if mixed_list:
        maskb_s = np.stack([
            m00[qt * 128:(qt + 1) * 128, kt * 128:(kt + 1) * 128]
            for qt, kt in mixed_list]).astype(MNP)

    shared = dict(
        xT=xTf.astype(QNP),
        woT=np.ascontiguousarray(wo.T).astype(BF16_NP),
        frT=np.ascontiguousarray(fr.T),
        fiT=np.ascontiguousarray(fi.T),
    )
    if qk_f32:
        shared["xTv"] = xTf.astype(BF16_NP)
    if maskb_s is not None:
        shared["maskb"] = maskb_s

    in_maps = []
    for c in range(N_CORES):
        wq_c = wq_p[c * HPC:(c + 1) * HPC].reshape(HPC * HD, D)
        wk_c = wk_p[c]
        wv_c = wv.reshape(KV, HD, D)[c]
        m = dict(shared)
        m["wqT"] = np.ascontiguousarray(wq_c.T).astype(QNP)
        m["wkT"] = np.ascontiguousarray(wk_c.T).astype(QNP)
        m["wvT"] = np.ascontiguousarray(wv_c.T).astype(BF16_NP)
        in_maps.append(m)
    return in_maps, qk_f32, blocks, jlim, mask_f32, exp_shift


_PROGRAM_CACHE = {}


def run(inputs, trace=False):
    in_maps, qk_f32, blocks, jlim, mask_f32, exp_shift = _prep(inputs)
    key = (qk_f32, blocks.tobytes(), jlim.tobytes(), mask_f32, exp_shift)
    if key not in _PROGRAM_CACHE:
        _PROGRAM_CACHE[key] = build_program(qk_f32, blocks, jlim, mask_f32,
                                            exp_shift)
    nc = _PROGRAM_CACHE[key]
    res = run_bass_kernel_spmd(nc, in_maps, core_ids=list(range(N_CORES)),
                               trace=trace)
    out = np.empty((B, S, D), np.float32)
    xk = np.empty((B, S, KV, HD), np.float32)
    xv = np.empty((B, S, KV, HD), np.float32)
    for c in range(N_CORES):
        r = res.results[c]
        out[:, c * SC:(c + 1) * SC, :] = r["out"].reshape(B, SC, D)
        xk[:, :, c, :] = r["xk"].reshape(B, S, HD)
        xv[:, :, c, :] = r["xv"].reshape(B, S, HD)
    return (out, xk, xv), res


def kernel(**inputs):
    (out, xk, xv), _ = run(inputs, trace=False)
    return out, xk, xv
